# revision 11
# baseline (speedup 1.0000x reference)
"""Trainium2 Bass kernel for nn_CrossAttention (B=4, C=384, H=W=64, n_div=12).

Sharding: 8 cores = 4 batch samples x 2 query-row halves. Each core computes
cross-attention for a 34-row query window (32 output rows + 1 halo row each
side for the 3x3 conv; out-of-image halo rows masked to zero), the conv,
bias and residual for its 32 output rows. No collectives.

Per-core pipeline (bf16 matmuls, fp32 residual):
  KV = Wkv @ xe                        [64, 4096] -> K,V bf16
  Q  = (Wq/sqrt(32)) @ x (f32r) -> bf16 [32, 2176]
  V -> PE-transpose -> VT bf16         [128, 32, 33] (col 32 = ones)
  per query chunk (4 x 512 + 128), per key chunk (32 x 128), software-
  pipelined with a 2-iteration skew so the PE never waits on exp:
     S^T = K^T Q   bf16 matmul          [128, 512]
     E   = exp(S^T)  alternating ACT Exp / DVE Schraudolph  (bf16)
     acc += VT^T E   bf16 matmul        [33, 512]
  attn = acc[0:32] * rmask * recip_fast(acc[32]) broadcast via GPSIMD
  conv3x3 as 3 dx-shifted bf16 matmuls, bias folded in as a ones-row,
  residual add (f32) on DVE, DMA out.
"""

import math
import os
from contextlib import ExitStack

import numpy as np
import ml_dtypes

B, C, H, W = 4, 384, 64, 64
ND = 12
D = C // ND                      # 32 projected channels
SCALE = 1.0 / math.sqrt(D)
NCORES = 8
WROWS = 34                       # query window rows (32 out + 2 halo slots)
NQ = WROWS * W                   # 2176 query positions per core
NK = H * W                       # 4096 key/value positions
NYX = NK // 128                  # 32 key chunks
HW_CHUNKS = [(0, 512), (512, 512), (1024, 512), (1536, 512), (2048, 128)]
NOUT = 32 * W                    # 2048 output positions per core

LOG2E = 1.4426950408889634
SCH_A = 128.0 * LOG2E            # Schraudolph scale (bf16 bit pattern)
SCH_B = 16256.0 - 5.625          # 127*128 - C, C calibrated numerically

_NC_CACHE = None
LAST_RESULTS = None


def _build_nc():
    import concourse.bass as bass
    import concourse.mybir as mybir
    import concourse.tile as tile
    from concourse import bacc
    from concourse.masks import make_identity

    f32 = mybir.dt.float32
    f32r = mybir.dt.float32r
    bf16 = mybir.dt.bfloat16
    i16 = mybir.dt.int16
    AF = mybir.ActivationFunctionType
    MUL = mybir.AluOpType.mult
    ADD = mybir.AluOpType.add

    nc = bacc.Bacc()
    xq_d = nc.declare_dram_parameter("xq", [C, NQ], f32r, isOutput=False)
    xe_d = nc.declare_dram_parameter("xe", [C, NK], bf16, isOutput=False)
    wqt_d = nc.declare_dram_parameter("wqt", [C, D], f32r, isOutput=False)
    wkvt_d = nc.declare_dram_parameter("wkvt", [C, 2 * D], bf16, isOutput=False)
    rmask_d = nc.declare_dram_parameter("rmask", [1, NQ], f32, isOutput=False)
    w3_d = nc.declare_dram_parameter("w3", [3, 3 * D + 1, C], bf16, isOutput=False)
    out_d = nc.declare_dram_parameter("out", [C, NOUT], f32, isOutput=True)

    with ExitStack() as ctx:
        tc = ctx.enter_context(tile.TileContext(nc))
        const = ctx.enter_context(tc.tile_pool(name="const", bufs=1))
        big = ctx.enter_context(tc.tile_pool(name="big", bufs=1))
        exl = ctx.enter_context(tc.tile_pool(name="exl", bufs=6))
        outl = ctx.enter_context(tc.tile_pool(name="outl", bufs=3))
        ps_tmp = ctx.enter_context(tc.tile_pool(name="ps_tmp", bufs=2, space="PSUM"))
        ps_st = ctx.enter_context(tc.tile_pool(name="ps_st", bufs=4, space="PSUM"))
        ps_acc = ctx.enter_context(tc.tile_pool(name="ps_acc", bufs=2, space="PSUM"))

        # ---------------- weights / constants ----------------
        wq_sb = const.tile([128, 3, D], f32r)
        nc.gpsimd.dma_start(out=wq_sb, in_=wqt_d[:, :].rearrange("(c p) d -> p c d", p=128))
        wkv_sb = const.tile([128, 3, 2 * D], bf16)
        nc.gpsimd.dma_start(out=wkv_sb, in_=wkvt_d[:, :].rearrange("(c p) d -> p c d", p=128))
        w3_sb = const.tile([3 * D + 1, 3, C], bf16)
        nc.gpsimd.dma_start(out=w3_sb, in_=w3_d[:, :, :].rearrange("k p o -> p k o"))
        rmask_sb = const.tile([1, NQ], f32)
        nc.gpsimd.dma_start(out=rmask_sb, in_=rmask_d[:, :])
        identb = const.tile([D, D], bf16)
        make_identity(nc, identb)

        # ---------------- activations in ----------------
        # xe and xq alternate across the two hwdge queues (sync + scalar).
        xe_sb = big.tile([128, 3, NK], bf16)
        xe_r = xe_d[:, :].rearrange("(c p) n -> p c n", p=128)
        for t in range(8):
            sl = slice(t * 512, (t + 1) * 512)
            eng = nc.sync if t % 2 == 0 else nc.scalar
            eng.dma_start(out=xe_sb[:, :, sl], in_=xe_r[:, :, sl])
        xq_sb = big.tile([128, 3, NQ], f32r)
        xq_r = xq_d[:, :].rearrange("(c p) n -> p c n", p=128)
        for i, (h0, hsz) in enumerate(HW_CHUNKS):
            sl = slice(h0, h0 + hsz)
            eng = nc.sync if i % 2 == 0 else nc.scalar
            eng.dma_start(out=xq_sb[:, :, sl], in_=xq_r[:, :, sl])

        qb_sb = big.tile([D, NQ], bf16)
        kb_sb = big.tile([D, NK], bf16)
        vb_sb = big.tile([D, NK], bf16)
        vt_sb = big.tile([128, NYX, D + 1], bf16)
        m_sb = big.tile([D, NQ], bf16)
        a3 = big.tile([3 * D + 1, 32, 66], bf16)
        dd = big.tile([1, NQ], f32)
        rr = big.tile([1, NQ], f32)
        rm = big.tile([1, NQ], f32)
        rb = big.tile([D, NQ], f32)

        # ---------------- projections ----------------
        # K/V: [64, 4096] = Wkv @ xe (bf16), 8 n-tiles of 512
        for t in range(8):
            sl = slice(t * 512, (t + 1) * 512)
            kvp = ps_tmp.tile([2 * D, 512], f32, tag="tmp")
            for c in range(3):
                nc.tensor.matmul(
                    kvp, wkv_sb[:, c, :], xe_sb[:, c, sl],
                    start=(c == 0), stop=(c == 2),
                )
            nc.scalar.activation(out=kb_sb[:, sl], in_=kvp[0:D, :], func=AF.Copy)
            nc.vector.tensor_copy(out=vb_sb[:, sl], in_=kvp[D:2 * D, :])

        # Q: [32, 2176] = (Wq*scale) @ x (f32r) -> bf16
        for h0, hsz in HW_CHUNKS:
            sl = slice(h0, h0 + hsz)
            qp = ps_tmp.tile([D, 512], f32, tag="tmp")
            for c in range(3):
                nc.tensor.matmul(
                    qp[:, :hsz], wq_sb[:, c, :], xq_sb[:, c, sl],
                    start=(c == 0), stop=(c == 2),
                )
            nc.scalar.activation(out=qb_sb[:, sl], in_=qp[:, :hsz], func=AF.Copy)

        # VT: [128, yx, 33]; col D is all-ones (denominator row)
        nc.vector.memset(vt_sb[:, :, D:D + 1], 1.0)
        for g in range(8):
            tp = ps_tmp.tile([128, 4 * D], bf16, tag="tmp")
            for j in range(4):
                yx = g * 4 + j
                nc.tensor.transpose(
                    tp[:, j * D:(j + 1) * D],
                    vb_sb[:, yx * 128:(yx + 1) * 128], identb,
                )
            nc.vector.tensor_copy(
                out=vt_sb[:, g * 4:(g + 1) * 4, 0:D],
                in_=tp[:, :].rearrange("p (j d) -> p j d", j=4),
            )

        # ---------------- attention (2-deep software pipeline) ----------
        def emit_st(hi, yx):
            h0, hsz = HW_CHUNKS[hi]
            sl = slice(h0, h0 + hsz)
            st = ps_st.tile([128, 512], f32, tag="st")
            nc.tensor.matmul(
                st[:, :hsz], kb_sb[:, yx * 128:(yx + 1) * 128],
                qb_sb[:, sl], start=True, stop=True,
            )
            ex = exl.tile([128, 512], i16, tag="ex")
            if yx % 2 == 0:
                nc.scalar.activation(
                    out=ex.bitcast(bf16)[:, :hsz], in_=st[:, :hsz], func=AF.Exp,
                )
            else:
                nc.vector.tensor_scalar(
                    out=ex[:, :hsz], in0=st[:, :hsz],
                    scalar1=SCH_A, scalar2=SCH_B, op0=MUL, op1=ADD,
                )
            return ex

        def emit_av(acc, hsz, yx, ex):
            nc.tensor.matmul(
                acc[:, :hsz], vt_sb[:, yx, :], ex.bitcast(bf16)[:, :hsz],
                start=(yx == 0), stop=(yx == NYX - 1),
            )

        # a3[(dy*32+i), ro, cc] = m[i, ro+dy, cc-1], zero padded at cc=0,65;
        # row 96 = 1.0 everywhere (bias via w3[1, 96, :] = bout)
        nc.gpsimd.memset(a3[0:3 * D, :, 0:1].bitcast(i16), 0)
        nc.gpsimd.memset(a3[0:3 * D, :, 65:66].bitcast(i16), 0)
        nc.vector.memset(a3[3 * D:3 * D + 1, :, :], 1.0)
        m_v = m_sb[:, :].rearrange("p (r w) -> p r w", w=W)

        def emit_div(hi, acc):
            # finalize chunk hi (m rows 8*hi .. 8*hi+8, last chunk rows 32-33)
            h0, hsz = HW_CHUNKS[hi]
            sl = slice(h0, h0 + hsz)
            nc.vector.tensor_copy(out=dd[:, sl], in_=acc[D:D + 1, :hsz])
            nc.vector.reciprocal_approx_fast(out=rr[:, sl], in_=dd[:, sl])
            nc.vector.tensor_mul(rm[:, sl], rr[:, sl], rmask_sb[:, sl])
            nc.gpsimd.partition_broadcast(rb[:, sl], rm[:, sl])
            nc.vector.tensor_mul(m_sb[:, sl], acc[0:D, :hsz], rb[:, sl])
            # stage finalized m rows into the a3 dy-bands
            r0, r1 = 8 * hi, min(8 * hi + 8, WROWS)
            for dy in range(3):
                a, b = max(r0 - dy, 0), min(r1 - dy, 32)
                if a >= b:
                    continue
                nc.sync.dma_start(
                    out=a3[D * dy:D * (dy + 1), a:b, 1:65],
                    in_=m_v[:, a + dy:b + dy, :],
                )

        def emit_conv(rg):
            for oc in range(3):
                osl = slice(oc * 128, (oc + 1) * 128)
                cp = ps_tmp.tile([128, 512], f32, tag="tmp")
                for dx in range(3):
                    nc.tensor.matmul(
                        cp, w3_sb[:, dx, osl],
                        a3[:, rg * 8:(rg + 1) * 8, dx:dx + 64],
                        start=(dx == 0), stop=(dx == 2),
                    )
                ot = outl.tile([128, 512], f32, tag="ot")
                res = xq_sb[:, oc, W + rg * 512: W + (rg + 1) * 512].bitcast(f32)
                nc.vector.tensor_add(ot, cp, res)
                eng = nc.sync if oc % 2 == 0 else nc.scalar
                eng.dma_start(
                    out=out_d[osl, rg * 512:(rg + 1) * 512], in_=ot,
                )

        # 2-deep software pipeline; chunk hi's division is emitted early in
        # chunk hi+1, and conv row-group rg (needs chunks rg, rg+1) is
        # emitted once both are divided.
        SKEW = 2
        pend = []          # [(acc, hsz, yx, ex)] waiting for AV emission
        div_pend = []      # [(hi, acc)] waiting for division emission
        conv_pend = 0      # next conv rg to emit
        divs_done = 0
        for hi in range(len(HW_CHUNKS)):
            hsz = HW_CHUNKS[hi][1]
            acc = ps_acc.tile([D + 1, 512], f32, tag="acc")
            for yx in range(NYX):
                ex = emit_st(hi, yx)
                pend.append((acc, hsz, yx, ex))
                if len(pend) > SKEW:
                    emit_av(*pend.pop(0))
                if yx == 3 and div_pend:
                    emit_div(*div_pend.pop(0))
                    divs_done += 1
                if yx == 8 and conv_pend < 4 and divs_done >= conv_pend + 2:
                    emit_conv(conv_pend)
                    conv_pend += 1
            div_pend.append((hi, acc))
        while pend:
            emit_av(*pend.pop(0))
        while div_pend:
            emit_div(*div_pend.pop(0))
            divs_done += 1
        while conv_pend < 4:
            emit_conv(conv_pend)
            conv_pend += 1

    if not nc.is_finalized():
        nc.finalize()
    return nc


def _make_in_maps(inputs):
    x = np.ascontiguousarray(np.asarray(inputs["x"], dtype=np.float32))
    xe = np.ascontiguousarray(np.asarray(inputs["xe"], dtype=np.float32))
    Wq = np.asarray(inputs["Wq"], dtype=np.float32)
    Wkv = np.asarray(inputs["Wkv"], dtype=np.float32)
    Wout = np.asarray(inputs["Wout"], dtype=np.float32)
    bout = np.asarray(inputs["bout"], dtype=np.float32)

    wqt = np.ascontiguousarray(Wq.T * SCALE)                  # [384, 32]
    wkvt = np.ascontiguousarray(Wkv.T).astype(ml_dtypes.bfloat16)  # [384, 64]
    # w3[dx, dy*32+i, o] = Wout[o, i, dy, dx]; row 96 = bias (dx=1 only)
    w3 = np.zeros((3, 3 * D + 1, C), dtype=np.float32)
    w3[:, :3 * D, :] = Wout.transpose(3, 2, 1, 0).reshape(3, 3 * D, C)
    w3[1, 3 * D, :] = bout
    w3 = w3.astype(ml_dtypes.bfloat16)

    in_maps = []
    for core in range(NCORES):
        b = core // 2
        top = (core % 2 == 0)
        xq = np.zeros((C, WROWS, W), dtype=np.float32)
        rmask = np.ones((1, WROWS, W), dtype=np.float32)
        if top:
            xq[:, 1:34, :] = x[b][:, 0:33, :]
            rmask[0, 0, :] = 0.0
        else:
            xq[:, 0:33, :] = x[b][:, 31:64, :]
            rmask[0, 33, :] = 0.0
        in_maps.append({
            "xq": np.ascontiguousarray(xq.reshape(C, NQ)),
            "xe": np.ascontiguousarray(xe[b].reshape(C, NK)).astype(ml_dtypes.bfloat16),
            "wqt": wqt,
            "wkvt": wkvt,
            "rmask": np.ascontiguousarray(rmask.reshape(1, NQ)),
            "w3": w3,
        })
    return in_maps


def _gather(results):
    out = np.empty((B, C, H, W), dtype=np.float32)
    for core in range(NCORES):
        b = core // 2
        rh = 0 if core % 2 == 0 else 32
        out[b, :, rh:rh + 32, :] = results[core]["out"].reshape(C, 32, W)
    return out


def kernel(**inputs) -> np.ndarray:
    global _NC_CACHE, LAST_RESULTS
    from concourse.bass_utils import run_bass_kernel_spmd

    if _NC_CACHE is None:
        _NC_CACHE = _build_nc()
    in_maps = _make_in_maps(inputs)
    tmpdir = os.environ.get("BASS_TRACE_TMPDIR") or None
    if tmpdir:
        os.makedirs(tmpdir, exist_ok=True)
    res = run_bass_kernel_spmd(_NC_CACHE, in_maps, list(range(NCORES)), tmpdir=tmpdir)
    LAST_RESULTS = res
    return _gather(res.results)


# revision 15
# speedup vs baseline: 1.0065x; 1.0065x over previous
"""Trainium2 Bass kernel for nn_CrossAttention (B=4, C=384, H=W=64, n_div=12).

Sharding: 8 cores = 4 batch samples x 2 query-row halves. Each core computes
cross-attention for a 34-row query window (32 output rows + 1 halo row each
side for the 3x3 conv; out-of-image halo rows masked to zero), the conv,
bias and residual for its 32 output rows. No collectives.

Per-core pipeline (bf16 matmuls, fp32 residual):
  KV = Wkv @ xe                        [64, 4096] -> K,V bf16
  Q  = (Wq/sqrt(32)) @ x (f32r) -> bf16 [32, 2176]
  V -> PE-transpose -> VT bf16         [128, 32, 33] (col 32 = ones)
  per query chunk (4 x 512 + 128), per key chunk (32 x 128), software-
  pipelined with a 2-iteration skew so the PE never waits on exp:
     S^T = K^T Q   bf16 matmul          [128, 512]
     E   = exp(S^T)  alternating ACT Exp / DVE Schraudolph  (bf16)
     acc += VT^T E   bf16 matmul        [33, 512]
  attn = acc[0:32] * rmask * recip_fast(acc[32]) broadcast via GPSIMD
  conv3x3 as 3 dx-shifted bf16 matmuls, bias folded in as a ones-row,
  residual add (f32) on DVE, DMA out.
"""

import math
import os
from contextlib import ExitStack

import numpy as np
import ml_dtypes

B, C, H, W = 4, 384, 64, 64
ND = 12
D = C // ND                      # 32 projected channels
SCALE = 1.0 / math.sqrt(D)
NCORES = 8
WROWS = 34                       # query window rows (32 out + 2 halo slots)
NQ = WROWS * W                   # 2176 query positions per core
NK = H * W                       # 4096 key/value positions
NYX = NK // 128                  # 32 key chunks
HW_CHUNKS = [(0, 512), (512, 512), (1024, 512), (1536, 512), (2048, 128)]
NOUT = 32 * W                    # 2048 output positions per core

LOG2E = 1.4426950408889634
SCH_A = 128.0 * LOG2E            # Schraudolph scale (bf16 bit pattern)
SCH_B = 16256.0 - 5.625          # 127*128 - C, C calibrated numerically

_NC_CACHE = None
LAST_RESULTS = None


def _build_nc():
    import concourse.bass as bass
    import concourse.mybir as mybir
    import concourse.tile as tile
    from concourse import bacc
    from concourse.masks import make_identity

    f32 = mybir.dt.float32
    f32r = mybir.dt.float32r
    bf16 = mybir.dt.bfloat16
    i16 = mybir.dt.int16
    AF = mybir.ActivationFunctionType
    MUL = mybir.AluOpType.mult
    ADD = mybir.AluOpType.add

    nc = bacc.Bacc()
    xq_d = nc.declare_dram_parameter("xq", [C, NQ], f32r, isOutput=False)
    xe_d = nc.declare_dram_parameter("xe", [C, NK], bf16, isOutput=False)
    wqt_d = nc.declare_dram_parameter("wqt", [C, D], f32r, isOutput=False)
    wkvt_d = nc.declare_dram_parameter("wkvt", [C, 2 * D], bf16, isOutput=False)
    rmask_d = nc.declare_dram_parameter("rmask", [1, NQ], f32, isOutput=False)
    w3_d = nc.declare_dram_parameter("w3", [3, 3 * D + 1, C], bf16, isOutput=False)
    out_d = nc.declare_dram_parameter("out", [C, NOUT], f32, isOutput=True)

    with ExitStack() as ctx:
        tc = ctx.enter_context(tile.TileContext(nc))
        const = ctx.enter_context(tc.tile_pool(name="const", bufs=1))
        big = ctx.enter_context(tc.tile_pool(name="big", bufs=1))
        exl = ctx.enter_context(tc.tile_pool(name="exl", bufs=6))
        outl = ctx.enter_context(tc.tile_pool(name="outl", bufs=3))
        ps_tmp = ctx.enter_context(tc.tile_pool(name="ps_tmp", bufs=2, space="PSUM"))
        ps_st = ctx.enter_context(tc.tile_pool(name="ps_st", bufs=4, space="PSUM"))
        ps_acc = ctx.enter_context(tc.tile_pool(name="ps_acc", bufs=2, space="PSUM"))

        # ---------------- weights / constants ----------------
        wq_sb = const.tile([128, 3, D], f32r)
        nc.gpsimd.dma_start(out=wq_sb, in_=wqt_d[:, :].rearrange("(c p) d -> p c d", p=128))
        wkv_sb = const.tile([128, 3, 2 * D], bf16)
        nc.gpsimd.dma_start(out=wkv_sb, in_=wkvt_d[:, :].rearrange("(c p) d -> p c d", p=128))
        w3_sb = const.tile([3 * D + 1, 3, C], bf16)
        nc.gpsimd.dma_start(out=w3_sb, in_=w3_d[:, :, :].rearrange("k p o -> p k o"))
        rmask_sb = const.tile([1, NQ], f32)
        nc.gpsimd.dma_start(out=rmask_sb, in_=rmask_d[:, :])
        identb = const.tile([D, D], bf16)
        make_identity(nc, identb)

        # ---------------- activations in ----------------
        # xe and xq alternate across the two hwdge queues (sync + scalar).
        xe_sb = big.tile([128, 3, NK], bf16)
        xe_r = xe_d[:, :].rearrange("(c p) n -> p c n", p=128)
        for t in range(8):
            sl = slice(t * 512, (t + 1) * 512)
            eng = nc.sync if t % 2 == 0 else nc.scalar
            eng.dma_start(out=xe_sb[:, :, sl], in_=xe_r[:, :, sl])
        xq_sb = big.tile([128, 3, NQ], f32r)
        xq_r = xq_d[:, :].rearrange("(c p) n -> p c n", p=128)
        for i, (h0, hsz) in enumerate(HW_CHUNKS):
            sl = slice(h0, h0 + hsz)
            eng = nc.sync if i % 2 == 0 else nc.scalar
            eng.dma_start(out=xq_sb[:, :, sl], in_=xq_r[:, :, sl])

        qb_sb = big.tile([D, NQ], bf16)
        kb_sb = big.tile([D, NK], bf16)
        vb_sb = big.tile([D, NK], bf16)
        vt_sb = big.tile([128, NYX, D + 1], bf16)
        m_sb = big.tile([D, NQ], bf16)
        a3 = big.tile([3 * D + 1, 32, 66], bf16)
        dd = big.tile([1, NQ], f32)
        rr = big.tile([1, NQ], f32)
        rm = big.tile([1, NQ], f32)
        rb = big.tile([D, NQ], f32)

        # ---------------- projections ----------------
        # K/V: [64, 4096] = Wkv @ xe (bf16), 8 n-tiles of 512
        for t in range(8):
            sl = slice(t * 512, (t + 1) * 512)
            kvp = ps_tmp.tile([2 * D, 512], f32, tag="tmp")
            for c in range(3):
                nc.tensor.matmul(
                    kvp, wkv_sb[:, c, :], xe_sb[:, c, sl],
                    start=(c == 0), stop=(c == 2),
                )
            nc.scalar.activation(out=kb_sb[:, sl], in_=kvp[0:D, :], func=AF.Copy)
            nc.vector.tensor_copy(out=vb_sb[:, sl], in_=kvp[D:2 * D, :])

        # Q: [32, 2176] = (Wq*scale) @ x (f32r) -> bf16
        for h0, hsz in HW_CHUNKS:
            sl = slice(h0, h0 + hsz)
            qp = ps_tmp.tile([D, 512], f32, tag="tmp")
            for c in range(3):
                nc.tensor.matmul(
                    qp[:, :hsz], wq_sb[:, c, :], xq_sb[:, c, sl],
                    start=(c == 0), stop=(c == 2),
                )
            nc.scalar.activation(out=qb_sb[:, sl], in_=qp[:, :hsz], func=AF.Copy)

        # VT: [128, yx, 33]; col D is all-ones (denominator row)
        nc.vector.memset(vt_sb[:, :, D:D + 1], 1.0)
        for g in range(8):
            tp = ps_tmp.tile([128, 4 * D], bf16, tag="tmp")
            for j in range(4):
                yx = g * 4 + j
                nc.tensor.transpose(
                    tp[:, j * D:(j + 1) * D],
                    vb_sb[:, yx * 128:(yx + 1) * 128], identb,
                )
            nc.vector.tensor_copy(
                out=vt_sb[:, g * 4:(g + 1) * 4, 0:D],
                in_=tp[:, :].rearrange("p (j d) -> p j d", j=4),
            )

        # ---------------- attention (2-deep software pipeline) ----------
        def emit_st(hi, yx):
            h0, hsz = HW_CHUNKS[hi]
            sl = slice(h0, h0 + hsz)
            st = ps_st.tile([128, 512], f32, tag="st")
            nc.tensor.matmul(
                st[:, :hsz], kb_sb[:, yx * 128:(yx + 1) * 128],
                qb_sb[:, sl], start=True, stop=True,
            )
            ex = exl.tile([128, 512], i16, tag="ex")
            if yx % 2 == 0:
                nc.scalar.activation(
                    out=ex.bitcast(bf16)[:, :hsz], in_=st[:, :hsz], func=AF.Exp,
                )
            else:
                nc.vector.tensor_scalar(
                    out=ex[:, :hsz], in0=st[:, :hsz],
                    scalar1=SCH_A, scalar2=SCH_B, op0=MUL, op1=ADD,
                )
            return ex

        def emit_av(acc, hsz, yx, ex):
            nc.tensor.matmul(
                acc[:, :hsz], vt_sb[:, yx, :], ex.bitcast(bf16)[:, :hsz],
                start=(yx == 0), stop=(yx == NYX - 1),
            )

        # a3[(dy*32+i), ro, cc] = m[i, ro+dy, cc-1], zero padded at cc=0,65;
        # row 96 = 1.0 everywhere (bias via w3[1, 96, :] = bout)
        nc.gpsimd.memset(a3[0:3 * D, :, 0:1].bitcast(i16), 0)
        nc.gpsimd.memset(a3[0:3 * D, :, 65:66].bitcast(i16), 0)
        nc.vector.memset(a3[3 * D:3 * D + 1, :, :], 1.0)
        m_v = m_sb[:, :].rearrange("p (r w) -> p r w", w=W)

        def emit_div(hi, acc):
            # finalize chunk hi (m rows 8*hi .. 8*hi+8, last chunk rows 32-33)
            h0, hsz = HW_CHUNKS[hi]
            sl = slice(h0, h0 + hsz)
            nc.vector.tensor_copy(out=dd[:, sl], in_=acc[D:D + 1, :hsz])
            nc.vector.reciprocal_approx_fast(out=rr[:, sl], in_=dd[:, sl])
            nc.vector.tensor_mul(rm[:, sl], rr[:, sl], rmask_sb[:, sl])
            nc.gpsimd.partition_broadcast(rb[:, sl], rm[:, sl])
            nc.vector.tensor_mul(m_sb[:, sl], acc[0:D, :hsz], rb[:, sl])
            # stage finalized m rows into the a3 dy-bands
            r0, r1 = 8 * hi, min(8 * hi + 8, WROWS)
            for dy in range(3):
                a, b = max(r0 - dy, 0), min(r1 - dy, 32)
                if a >= b:
                    continue
                nc.sync.dma_start(
                    out=a3[D * dy:D * (dy + 1), a:b, 1:65],
                    in_=m_v[:, a + dy:b + dy, :],
                )

        def emit_conv(rg):
            for oc in range(3):
                osl = slice(oc * 128, (oc + 1) * 128)
                cp = ps_tmp.tile([128, 512], f32, tag="tmp")
                for dx in range(3):
                    nc.tensor.matmul(
                        cp, w3_sb[:, dx, osl],
                        a3[:, rg * 8:(rg + 1) * 8, dx:dx + 64],
                        start=(dx == 0), stop=(dx == 2),
                    )
                ot = outl.tile([128, 512], f32, tag="ot")
                res = xq_sb[:, oc, W + rg * 512: W + (rg + 1) * 512].bitcast(f32)
                nc.vector.tensor_add(ot, cp, res)
                eng = nc.sync if oc % 2 == 0 else nc.scalar
                eng.dma_start(
                    out=out_d[osl, rg * 512:(rg + 1) * 512], in_=ot,
                )

        # 2-deep software pipeline; chunk hi's division is emitted early in
        # chunk hi+1, and conv row-group rg (needs chunks rg, rg+1) is
        # emitted once both are divided.
        SKEW = 2
        pend = []          # [(acc, hsz, yx, ex)] waiting for AV emission
        div_pend = []      # [(hi, acc)] waiting for division emission
        conv_pend = 0      # next conv rg to emit
        divs_done = 0
        for hi in range(len(HW_CHUNKS)):
            hsz = HW_CHUNKS[hi][1]
            acc = ps_acc.tile([D + 1, 512], f32, tag="acc")
            for yx in range(NYX):
                ex = emit_st(hi, yx)
                pend.append((acc, hsz, yx, ex))
                if len(pend) > SKEW:
                    emit_av(*pend.pop(0))
                if yx == 3 and div_pend:
                    emit_div(*div_pend.pop(0))
                    divs_done += 1
                if yx == 8 and conv_pend < 4 and divs_done >= conv_pend + 2:
                    emit_conv(conv_pend)
                    conv_pend += 1
            div_pend.append((hi, acc))
        while pend:
            emit_av(*pend.pop(0))
        while div_pend:
            emit_div(*div_pend.pop(0))
            divs_done += 1
        while conv_pend < 4:
            emit_conv(conv_pend)
            conv_pend += 1

    if not nc.is_finalized():
        nc.finalize()
    return nc


def _make_in_maps(inputs):
    x = np.ascontiguousarray(np.asarray(inputs["x"], dtype=np.float32))
    xe = np.ascontiguousarray(np.asarray(inputs["xe"], dtype=np.float32))
    Wq = np.asarray(inputs["Wq"], dtype=np.float32)
    Wkv = np.asarray(inputs["Wkv"], dtype=np.float32)
    Wout = np.asarray(inputs["Wout"], dtype=np.float32)
    bout = np.asarray(inputs["bout"], dtype=np.float32)

    wqt = np.ascontiguousarray(Wq.T * SCALE)                  # [384, 32]
    wkvt = np.ascontiguousarray(Wkv.T).astype(ml_dtypes.bfloat16)  # [384, 64]
    # w3[dx, dy*32+i, o] = Wout[o, i, dy, dx]; row 96 = bias (dx=1 only)
    w3 = np.zeros((3, 3 * D + 1, C), dtype=np.float32)
    w3[:, :3 * D, :] = Wout.transpose(3, 2, 1, 0).reshape(3, 3 * D, C)
    w3[1, 3 * D, :] = bout
    w3 = w3.astype(ml_dtypes.bfloat16)

    in_maps = []
    for core in range(NCORES):
        b = core // 2
        top = (core % 2 == 0)
        xq = np.zeros((C, WROWS, W), dtype=np.float32)
        rmask = np.ones((1, WROWS, W), dtype=np.float32)
        if top:
            xq[:, 1:34, :] = x[b][:, 0:33, :]
            rmask[0, 0, :] = 0.0
        else:
            xq[:, 0:33, :] = x[b][:, 31:64, :]
            rmask[0, 33, :] = 0.0
        in_maps.append({
            "xq": np.ascontiguousarray(xq.reshape(C, NQ)),
            "xe": np.ascontiguousarray(xe[b].reshape(C, NK)).astype(ml_dtypes.bfloat16),
            "wqt": wqt,
            "wkvt": wkvt,
            "rmask": np.ascontiguousarray(rmask.reshape(1, NQ)),
            "w3": w3,
        })
    return in_maps


def _gather(results):
    out = np.empty((B, C, H, W), dtype=np.float32)
    for core in range(NCORES):
        b = core // 2
        rh = 0 if core % 2 == 0 else 32
        out[b, :, rh:rh + 32, :] = results[core]["out"].reshape(C, 32, W)
    return out


def kernel(**inputs) -> np.ndarray:
    global _NC_CACHE, LAST_RESULTS
    from concourse.bass_utils import run_bass_kernel_spmd

    if _NC_CACHE is None:
        _NC_CACHE = _build_nc()
    in_maps = _make_in_maps(inputs)
    tmpdir = os.environ.get("BASS_TRACE_TMPDIR") or None
    if tmpdir:
        os.makedirs(tmpdir, exist_ok=True)
    res = run_bass_kernel_spmd(_NC_CACHE, in_maps, list(range(NCORES)), tmpdir=tmpdir)
    LAST_RESULTS = res
    return _gather(res.results)
